# revision 30
# baseline (speedup 1.0000x reference)
"""MoE grouped-GEMM expert FFN (SwiGLU) for Trainium2, 8-core expert parallelism.

Contract: kernel(**inputs) takes FULL unsharded inputs, returns FULL output.

Strategy:
  - Host-side routing: tokens are contiguous per expert; split expert groups
    into chunks, band-assign chunks across 8 cores with an identical
    segment-capacity structure on every core (SPMD: one Bass program).
  - Per core, per segment: local GEMM1 (x @ w1w3) -> SwiGLU -> GEMM2 (h @ w2).
  - Host-side combine: scatter per-core output rows back to full output.

Matmul dtype is float16 (PSUM/silu stay fp32): full PE column rate and half
the DMA bytes of fp32, ~5e-4 rel err.

Layout choices:
  - x: packed per token tile as [tile, 128, 8*tt] (hidden chunk k on the
    free dim); the first tile is loaded in three pieces (k0 | k1 | k2-7) so
    the first matmul only waits for ~0.5MB.
  - w1w3: columns permuted so psum chunk pairs hold (gate, up) 128-blocks
    (SwiGLU = full-width ACT/DVE ops); rows packed as [S, 4, 128, 2*1408]
    (k-chunk pairs); the first segment's first pair is split into two
    [128, 1408] tiles to shorten the critical path.
  - w2: rows packed as [S, 128, 6*1024] (inter chunk j on partitions, hidden
    on free dim); the 64-row tail chunk loads as a [64, 1024] DMA so no
    zero-pad bytes move.
  - GEMM1 iterates k (contraction) outer / m inner within m-groups of 2.
  - GEMM2 style is per-segment:
    * cap >= 256 (transposed): w2 [inter_c, hidden_p] stationary, h moving,
      j-outer over hidden-pair columns (moving operand reused 2x, ~48
      cols/token); output lands [hidden, token], stored fp16 with >=512B
      partition runs (DMA line rate), host transposes back.
    * cap < 256 (plain): h stationary, w2 moving (512-col streams); output
      [token, hidden] fp16 rows store with 2KB runs, so the small final
      segments drain at line rate.
"""

import numpy as np

import concourse.bacc as bacc
import concourse.mybir as mybir
from concourse import tile
from concourse.bass_utils import run_bass_kernel_spmd

HIDDEN = 1024
INTER = 704
N_EXPERTS = 32
NCORES = 8
KC = HIDDEN // 128  # 8 k-chunks over hidden
MC = (2 * INTER) // 128  # 11 m-chunks over permuted gate|up dim
JC = (INTER + 127) // 128  # 6 j-chunks over inter for GEMM2 (last is 64 rows)
PC = HIDDEN // 128  # 8 hidden chunks for transposed GEMM2 output
TT = 512  # token tile (moving free dim)
M_GROUPS = [(0, 2), (2, 4), (4, 6), (6, 8), (8, 10), (10, 11)]  # pair-sized m-groups
WARMUP_MM = 37
T_STYLE_MIN = 256  # transposed GEMM2 for caps >= this (store runs >= 512B)

f32 = mybir.dt.float32
MM_DT = mybir.dt.float16
NP_DT = np.float16
ESZ = 2

# Column permutation of w1w3's last dim (2*INTER): m-chunks come in
# (gate, up) pairs of full 128-row blocks so SwiGLU runs full-width
# [128, tt] ACT/DVE ops. chunk 2j = gate[128j:128j+128], chunk 2j+1 =
# up[128j:128j+128] for j<5; the last chunk holds the 64-row tails
# [gate[640:704]|up[640:704]].
_PERM = np.empty(2 * INTER, dtype=np.int64)
for _j in range(5):
    _PERM[256 * _j : 256 * _j + 128] = np.arange(128 * _j, 128 * _j + 128)
    _PERM[256 * _j + 128 : 256 * _j + 256] = INTER + np.arange(
        128 * _j, 128 * _j + 128
    )
_PERM[1280:1344] = np.arange(640, 704)
_PERM[1344:1408] = INTER + np.arange(640, 704)


def _g2_cols(c, is_last=False):
    """Effective GEMM2 PE columns for a segment of capacity c."""
    if not is_last:
        return 48 * max(c, 128)  # transposed style
    return 12 * -(-c // 128) * 512  # plain style, 512-col moving streams


def _make_chunks_split(counts, starts, keys):
    """Chunks from an explicit per-expert split-count vector keys[e]."""
    chunks = []
    for e in range(N_EXPERTS):
        n = int(counts[e])
        a = int(starts[e])
        if n <= 0:
            continue
        nparts = max(1, int(keys.get(e, 1)))
        base, rem = divmod(n, nparts)
        off = 0
        for p in range(nparts):
            ln = base + (1 if p < rem else 0)
            if ln > 0:
                chunks.append((ln, e, a + off))
                off += ln
    return chunks


def _caps_of(chunks):
    """Band caps for a sorted-desc chunk list."""
    S = -(-len(chunks) // NCORES)
    caps = []
    for s in range(S):
        band = chunks[NCORES * s : NCORES * (s + 1)]
        caps.append(max(8, ((band[0][0] + 3) // 4) * 4))
    return caps


def _plan_cost(caps):
    """Wall-time score: PE columns + DMA bytes, both per core."""
    S = len(caps)
    pe_cols = sum(88 * max(c, 128) + _g2_cols(c, i == len(caps) - 1) for i, c in enumerate(caps)) + WARMUP_MM * TT
    pe_t = pe_cols / 2.4e9 / 0.90
    w_bytes = S * (HIDDEN * 2 * INTER + INTER * HIDDEN) * ESZ
    io_bytes = sum(caps) * HIDDEN * (ESZ + ESZ)  # xt in + out fp16
    dma_t = (w_bytes + io_bytes) / 380e9
    return max(pe_t, dma_t) + 0.25 * min(pe_t, dma_t)


def _plan(counts):
    """Balance (expert, token-chunk) pieces across NCORES cores.

    Chunks are sorted by size and dealt in bands of 8 (one per core): slot s
    capacity = the largest chunk in band s. Searches per-expert split counts
    to trade segment count (weight DMA) against padding (PE columns).
    """
    starts = np.zeros(N_EXPERTS, dtype=np.int64)
    np.cumsum(counts[:-1], out=starts[1:])

    order = np.argsort(-counts)  # experts by size desc
    nz = int((counts > 0).sum())

    best = None

    def consider(keys):
        chunks = _make_chunks_split(counts, starts, keys)
        if not chunks:
            chunks = [(8, None, 0)]
        chunks.sort(key=lambda c: -c[0])
        caps = _caps_of(chunks)
        score = _plan_cost(caps)
        nonlocal best
        if best is None or score < best[0]:
            best = (score, chunks, caps)

    # Uniform tmax scan.
    for tmax in (4096, 2048, 1024, 768, 512, *range(128, 513, 8)):
        keys = {int(e): -(-int(counts[e]) // tmax) for e in range(N_EXPERTS)}
        consider(keys)

    # Split-vector search: distribute E extra splits among the top experts.
    for S in (4, 5, 6):
        E = 8 * S - nz
        if E < 0:
            continue
        top = [int(e) for e in order[:10] if counts[e] > 0]

        def rec(i, left, keys):
            if i == len(top) or left == 0:
                if left == 0:
                    consider(dict(keys))
                return
            e = top[i]
            for extra in range(min(left, 5), -1, -1):
                keys[e] = 1 + extra
                rec(i + 1, left - extra, keys)
            keys.pop(e, None)

        if E <= 12 and len(top) > 0:
            rec(0, E, {})

    _, chunks, caps = best
    S = len(caps)
    offs = np.concatenate([[0], np.cumsum(caps)[:-1]]).astype(np.int64)
    cap_total = int(sum(caps))

    assign = [[] for _ in range(NCORES)]
    for s in range(S):
        band = chunks[NCORES * s : NCORES * (s + 1)]
        for c in range(NCORES):
            if c < len(band):
                n, e, a = band[c]
                assign[c].append((e, a, n))
            else:
                assign[c].append((None, 0, 0))
    return assign, caps, offs, cap_total


def _tiles_of(caps):
    """Token tiles as (segment, t0, tt) in execution order."""
    out = []
    for s, C in enumerate(caps):
        for t0 in range(0, C, TT):
            out.append((s, t0, min(TT, C - t0)))
    return out


def _n_offs(caps):
    """Row offsets into out2 (plain-style output): last segment only."""
    S = len(caps)
    return {S - 1: 0}, max(caps[-1], 1)


def _build(S, caps, cap_total):
    """Build the SPMD Bass program for one core's segment structure."""
    nc = bacc.Bacc(
        "TRN2",
        target_bir_lowering=False,
        debug=False,
        enable_asserts=False,
        num_devices=NCORES,
    )

    tiles = _tiles_of(caps)
    NT = len(tiles)
    offs = np.concatenate([[0], np.cumsum(caps)[:-1]]).astype(np.int64)
    offs2, n2_total = _n_offs(caps)

    xt_d = nc.declare_dram_parameter("xt", [NT, 128, KC * TT], MM_DT, isOutput=False)
    w13_d = nc.declare_dram_parameter(
        "w13", [S, 4, 128, 2 * 2 * INTER], MM_DT, isOutput=False
    )
    w2_d = nc.declare_dram_parameter(
        "w2", [S, 128, JC * HIDDEN], MM_DT, isOutput=False
    )
    # Transposed-style output: [hidden chunk, 128, token] fp16.
    out_d = nc.declare_dram_parameter(
        "out", [PC, 128, cap_total], MM_DT, isOutput=True
    )
    # Plain-style output rows for the small segments.
    out2_d = nc.declare_dram_parameter("out2", [n2_total, HIDDEN], MM_DT,
                                       isOutput=True)

    with tile.TileContext(nc) as tc:
        with (
            tc.tile_pool(name="w13p", bufs=8) as w13p,
            tc.tile_pool(name="w2p", bufs=4) as w2p,
            tc.tile_pool(name="xtp", bufs=4) as xtp,
            tc.tile_pool(name="hp", bufs=12) as hp,
            tc.tile_pool(name="sgp", bufs=6) as sgp,
            tc.tile_pool(name="warmp", bufs=1) as warmp,
            tc.tile_pool(name="outp", bufs=6) as outp,
            tc.tile_pool(name="outp2", bufs=2) as outp2,
            tc.tile_pool(name="ps1", bufs=4, space="PSUM") as ps1,
            tc.tile_pool(name="ps2", bufs=4, space="PSUM") as ps2,
        ):
            # HAM warmup: the PE clock ramps to full speed after ~3us of
            # sustained matmul activity, and ANY idle gap drops it back to
            # 1.2GHz for the next ~3.4us. Burn 512-col matmuls until the
            # first segment's GEMM1 data (~1.4MB) has landed (~13.5us), so
            # the real stream starts at full clock and never stalls during
            # the DMA fill.
            warm_sb = warmp.tile([128, TT], MM_DT, tag="warm", name="warm_sb")
            nc.vector.memset(warm_sb[:], 0.0)
            warm_ps = ps2.tile([128, TT], f32, tag="po", name="warm_ps",
                               padded_shape=[128, TT])
            for _w in range(WARMUP_MM):
                nc.tensor.matmul(
                    warm_ps[:],
                    warm_sb[:, 0:128],
                    warm_sb[:],
                    start=True,
                    stop=True,
                )

            tix = 0
            w2_pre = {}
            for s in range(S):
                C = caps[s]
                assert C <= TT, "per-segment xt preload assumes single tile"
                off = int(offs[s])
                t_style = s < S - 1

                # For the first segment, issue the first token tile's xt DMA
                # ahead of the weights: the queue drains in issue order, and
                # the first matmul needs (xt, w13 pair 0).
                xt_first = None
                w13_t = []
                if s == 0:
                    # kp0 first (smaller than xt0; unblocks LDW of the first
                    # matmuls), then xt0, then the rest.
                    w13t = w13p.tile([128, 2 * 2 * INTER], MM_DT, tag="w13t",
                                     name=f"w13t{s}_0")
                    nc.sync.dma_start(out=w13t[:], in_=w13_d[s, 0])
                    w13_t.append(w13t)
                    tt0 = min(TT, C)
                    xt_first = xtp.tile([128, KC * tt0], MM_DT, tag="xtt",
                                        name="xtt0",
                                        padded_shape=[128, KC * TT])
                    nc.sync.dma_start(out=xt_first[:], in_=xt_d[0, :, 0 : KC * tt0])

                for kp in range(len(w13_t), 4):
                    w13t = w13p.tile([128, 2 * 2 * INTER], MM_DT, tag="w13t",
                                     name=f"w13t{s}_{kp}")
                    nc.sync.dma_start(out=w13t[:], in_=w13_d[s, kp])
                    w13_t.append(w13t)
                # w2 for segments 2+ is prefetched on the SCALAR queue
                # (emitted one segment ahead, below): removes 4.3MB from the
                # sync queue tail so late segments' w13 loads land in time.
                if s in w2_pre:
                    w2t = w2_pre.pop(s)
                else:
                    w2t = w2p.tile([128, JC * HIDDEN], MM_DT, tag="w2t",
                                   name=f"w2t{s}")
                    nc.sync.dma_start(
                        out=w2t[:, 0 : 5 * HIDDEN], in_=w2_d[s, :, 0 : 5 * HIDDEN]
                    )
                    nc.sync.dma_start(
                        out=w2t[0:64, 5 * HIDDEN : 6 * HIDDEN],
                        in_=w2_d[s, 0:64, 5 * HIDDEN : 6 * HIDDEN],
                    )
                if s >= 1 and s + 1 < S:
                    w2n = w2p.tile([128, JC * HIDDEN], MM_DT, tag="w2t",
                                   name=f"w2t{s + 1}")
                    nc.scalar.dma_start(
                        out=w2n[:, 0 : 5 * HIDDEN],
                        in_=w2_d[s + 1, :, 0 : 5 * HIDDEN],
                    )
                    nc.scalar.dma_start(
                        out=w2n[0:64, 5 * HIDDEN : 6 * HIDDEN],
                        in_=w2_d[s + 1, 0:64, 5 * HIDDEN : 6 * HIDDEN],
                    )
                    w2_pre[s + 1] = w2n

                def w13_ap(k, m, w13_t=w13_t):
                    base = (k % 2) * 2 * INTER + 128 * m
                    return w13_t[k // 2][:, base : base + 128]

                def w2t_ap(j, p, w2t=w2t):
                    jw = min(128, INTER - 128 * j)
                    base = j * HIDDEN + 128 * p
                    return w2t[0:jw, base : base + 128]

                def w2n_ap(j, nn, w2t=w2t):
                    jw = min(128, INTER - 128 * j)
                    base = j * HIDDEN + 512 * nn
                    return w2t[0:jw, base : base + 512]

                for t0 in range(0, C, TT):
                    tt = min(TT, C - t0)
                    if t0 == 0 and xt_first is not None:
                        xt_tile0 = xt_first

                        def xt_ap(k, xt_tile=xt_tile0, tt=tt):
                            return xt_tile[:, k * tt : (k + 1) * tt]
                    else:
                        xt_tile = xtp.tile([128, KC * tt], MM_DT, tag="xtt",
                                           name=f"xtt{tix}",
                                           padded_shape=[128, KC * TT])
                        nc.sync.dma_start(
                            out=xt_tile[:], in_=xt_d[tix, :, 0 : KC * tt]
                        )

                        def xt_ap(k, xt_tile=xt_tile, tt=tt):
                            return xt_tile[:, k * tt : (k + 1) * tt]

                    h_t = []
                    for j in range(JC):
                        jw = min(128, INTER - 128 * j)
                        ht = hp.tile([jw, tt], MM_DT, tag="ht", name=f"ht{tix}_{j}",
                                     padded_shape=[128, TT])
                        h_t.append(ht)

                    # GEMM1: k-outer within m-groups of 2 psum tiles.
                    for m_lo, m_hi in M_GROUPS:
                        pgs = {}
                        for m in range(m_lo, m_hi):
                            pgs[m] = ps1.tile([128, tt], f32, tag="pg",
                                              name=f"pg{m}",
                                              padded_shape=[128, TT])
                        for k in range(KC):
                            for m in range(m_lo, m_hi):
                                nc.tensor.matmul(
                                    pgs[m][:],
                                    w13_ap(k, m),
                                    xt_ap(k),
                                    start=(k == 0),
                                    stop=(k == KC - 1),
                                )
                        for m in range(m_lo, m_hi):
                            if m == MC - 1:
                                # tail chunk: [gate 64 | up 64] on partitions
                                sg = sgp.tile([64, tt], f32, tag="sg",
                                              name=f"sg{m}",
                                              padded_shape=[128, TT])
                                nc.scalar.activation(
                                    sg[:], pgs[m][0:64, :],
                                    mybir.ActivationFunctionType.Silu,
                                )
                                nc.vector.tensor_mul(
                                    h_t[JC - 1][0:64, :], sg[:],
                                    pgs[m][64:128, :],
                                )
                            elif m % 2 == 1:
                                sg = sgp.tile([128, tt], f32, tag="sg",
                                              name=f"sg{m}",
                                              padded_shape=[128, TT])
                                nc.scalar.activation(
                                    sg[:], pgs[m - 1][:],
                                    mybir.ActivationFunctionType.Silu,
                                )
                                nc.vector.tensor_mul(
                                    h_t[m // 2][:], sg[:], pgs[m][:]
                                )

                    store_eng = nc.sync if s == S - 1 else nc.gpsimd
                    if t_style:
                        # GEMM2 transposed: j-outer over hidden-pair columns;
                        # moving operand (h_t[j]) reused across the pair,
                        # psum tiles roll through the pool so copies overlap
                        # the next pair's matmuls.
                        for pp in range(0, PC, 2):
                            pos = {}
                            for p in (pp, pp + 1):
                                pos[p] = ps2.tile([128, tt], f32, tag="po",
                                                  name=f"po{p}",
                                                  padded_shape=[128, TT])
                            for j in range(JC):
                                for p in (pp, pp + 1):
                                    nc.tensor.matmul(
                                        pos[p][:],
                                        w2t_ap(j, p),
                                        h_t[j][:],
                                        start=(j == 0),
                                        stop=(j == JC - 1),
                                    )
                            for p in (pp, pp + 1):
                                ob = outp.tile([128, tt], MM_DT, tag="ob",
                                               name=f"ob{p}",
                                               padded_shape=[128, TT])
                                nc.vector.tensor_copy(ob[:], pos[p][:])
                                store_eng.dma_start(
                                    out=out_d[p, :, off + t0 : off + t0 + tt],
                                    in_=ob[:],
                                )
                    else:
                        # GEMM2 plain: h stationary, w2 moving (512-col
                        # streams); token-major rows store at line rate.
                        o2 = offs2[s]
                        for tc0 in range(0, tt, 128):
                            tw = min(128, tt - tc0)
                            pon = {}
                            for nn in range(2):
                                pon[nn] = ps2.tile([tw, 512], f32, tag="po",
                                                   name=f"pon{nn}",
                                                   padded_shape=[128, TT])
                            for j in range(JC):
                                for nn in range(2):
                                    nc.tensor.matmul(
                                        pon[nn][:],
                                        h_t[j][:, tc0 : tc0 + tw],
                                        w2n_ap(j, nn),
                                        start=(j == 0),
                                        stop=(j == JC - 1),
                                    )
                            ob = outp2.tile([tw, HIDDEN], MM_DT, tag="ob2",
                                            name="ob2",
                                            padded_shape=[128, HIDDEN])
                            for nn in range(2):
                                nc.vector.tensor_copy(
                                    ob[:, 512 * nn : 512 * (nn + 1)],
                                    pon[nn][:],
                                )
                                store_eng.dma_start(
                                    out=out2_d[
                                        o2 + t0 + tc0 : o2 + t0 + tc0 + tw,
                                        512 * nn : 512 * (nn + 1),
                                    ],
                                    in_=ob[:, 512 * nn : 512 * (nn + 1)],
                                )
                    tix += 1

    nc.compile()
    return nc


_BUILD_CACHE = {}


def _get_program(S, caps, cap_total):
    key = (S, tuple(caps))
    if key not in _BUILD_CACHE:
        _BUILD_CACHE[key] = _build(S, caps, cap_total)
    return _BUILD_CACHE[key]


def _pack_inputs(x, assign, caps, offs, cap_total, packed_w):
    """Build per-core input dicts matching the device layouts."""
    tiles = _tiles_of(caps)
    NT = len(tiles)
    S = len(caps)
    in_maps = []
    for c in range(NCORES):
        xt_c = np.zeros((HIDDEN, cap_total), dtype=NP_DT)
        w13_c = np.zeros((S, 4, 128, 2 * 2 * INTER), dtype=NP_DT)
        w2_c = np.zeros((S, 128, JC * HIDDEN), dtype=NP_DT)
        for s, (e, a, n) in enumerate(assign[c]):
            if e is None or n <= 0:
                continue
            o = int(offs[s])
            xt_c[:, o : o + n] = np.asarray(x[a : a + n, :], dtype=NP_DT).T
            w13_c[s] = packed_w["w13"][e]
            w2_c[s] = packed_w["w2"][e]
        xt_pack = np.zeros((NT, 128, KC * TT), dtype=NP_DT)
        for tix, (s, t0, tt) in enumerate(tiles):
            o = int(offs[s])
            blk = xt_c[:, o + t0 : o + t0 + tt]  # [1024, tt]
            xt_pack[tix, :, 0 : KC * tt] = (
                blk.reshape(KC, 128, tt).transpose(1, 0, 2).reshape(128, KC * tt)
            )
        in_maps.append({"xt": xt_pack, "w13": w13_c, "w2": w2_c})
    return in_maps


def _prep_weights(w1w3, w2):
    """Permute/pack weights once (shared across cores)."""
    w13_perm = np.asarray(w1w3[:, :, _PERM], dtype=NP_DT)  # [E, HIDDEN, 2*INTER]
    w13_pack = (
        w13_perm.reshape(N_EXPERTS, 4, 2, 128, 2 * INTER)
        .transpose(0, 1, 3, 2, 4)
        .reshape(N_EXPERTS, 4, 128, 2 * 2 * INTER)
    )
    w2p_all = np.zeros((N_EXPERTS, 768, HIDDEN), dtype=NP_DT)
    w2p_all[:, :INTER] = np.asarray(w2, dtype=NP_DT)
    w2_pack = (
        w2p_all.reshape(N_EXPERTS, JC, 128, HIDDEN)
        .transpose(0, 2, 1, 3)
        .reshape(N_EXPERTS, 128, JC * HIDDEN)
    )
    return {"w13": w13_pack, "w2": w2_pack}


def _run(x, tokens_per_expert, w1w3, w2, trace=False):
    x = np.ascontiguousarray(np.asarray(x, dtype=np.float32))
    counts = np.asarray(tokens_per_expert, dtype=np.int64).copy()
    w1w3 = np.asarray(w1w3, dtype=np.float32)
    w2 = np.asarray(w2, dtype=np.float32)

    T = x.shape[0]
    # Clip group sizes like ragged_dot: groups are consecutive; anything
    # beyond T is out of range.
    counts = np.maximum(counts, 0)
    cum = np.cumsum(counts)
    over = cum > T
    if over.any():
        first = int(np.argmax(over))
        prev = int(cum[first - 1]) if first > 0 else 0
        counts[first] = T - prev
        counts[first + 1 :] = 0

    assign, caps, offs, cap_total = _plan(counts)
    S = len(caps)
    nc = _get_program(S, caps, cap_total)

    packed_w = _prep_weights(w1w3, w2)
    in_maps = _pack_inputs(x, assign, caps, offs, cap_total, packed_w)

    extra = {}
    if trace:
        import os

        os.makedirs("/tmp/moe_prof", exist_ok=True)
        for f in os.listdir("/tmp/moe_prof"):
            os.unlink(os.path.join("/tmp/moe_prof", f))
        extra["tmpdir"] = "/tmp/moe_prof"
    res = run_bass_kernel_spmd(nc, in_maps, list(range(NCORES)), trace=trace, **extra)

    offs2, _ = _n_offs(caps)
    out_full = np.zeros((T, HIDDEN), dtype=np.float32)
    for c in range(NCORES):
        oc = res.results[c]["out"]  # [PC, 128, cap_total] fp16
        ocf = oc.reshape(HIDDEN, cap_total)
        oc2 = res.results[c]["out2"]  # [n2_total, HIDDEN] fp16
        for s, (e, a, n) in enumerate(assign[c]):
            if e is None or n <= 0:
                continue
            if s < len(caps) - 1:
                o = int(offs[s])
                out_full[a : a + n, :] = ocf[:, o : o + n].T.astype(np.float32)
            else:
                o2 = offs2[s]
                out_full[a : a + n, :] = oc2[o2 : o2 + n, :].astype(np.float32)
    return out_full, res


def kernel(x, tokens_per_expert, w1w3, w2, decoding=False, **_ignored):
    out, _ = _run(x, tokens_per_expert, w1w3, w2, trace=False)
    return out
